# revision 22
# baseline (speedup 1.0000x reference)
"""Trainium2 Bass kernel for nn_CustomLoss_19061064859882.

loss = CE(y_pred, y_true) - penalty/N, where the penalty uses
p1 = softmax(y_pred)[:, 0] and per-class weights from the label histogram.

Device/host split: the O(N*C) work — per-row sum_c exp(y_pred[r, c]) over the
128 classes — runs on the 8 NeuronCores (data-parallel over rows). The
remaining O(N) bookkeeping (log of the row-sums, picked-logit gather, label
bincount, per-class weighted sums, final scalar) is cheap vectorized numpy on
the host in float64.

Device pipeline (v5) — DMA-bound, dual-engine decode:
  * Each core's shard ships TRANSPOSED (classes on partitions, rows on the
    free dim) and split across two wire formats to cut HBM bytes ~25%:
      - U tiles (16384 rows): uint8 codes u = round((x*log2e + 16)*8), i.e.
        the logit on a 1/8-log2 grid. The otherwise-idle ScalarE decodes
        them with a single ACT pass: exp(u*ln2/8 - 16*ln2) = e^x, using the
        ACT instruction's free scale/bias affine. 1 elem/lane/cycle.
      - F tiles (16384 rows): fp16 logits decoded on the DVE via the
        Schraudolph bit trick at 4x perf mode: tensor_scalar computes
        i16 = int(x*1024*log2(e) + 15360); those bits reinterpreted as fp16
        are 2^(x*log2e).
    U and F tiles interleave in the DMA stream so both engines stay fed.
  * class-sum via TensorE: ones-column matmuls reduce over the partition
    (class) dim, 512 rows per matmul into PSUM fp32. A shifted-window
    stationary (W[:, 64-k:128-k], ones only in absolute column 64) routes
    512-row block k to PSUM partition k, all accumulating into one
    [64, 512] PSUM bank.
  * one DVE PSUM->SBUF copy (fp32 -> fp16), one 64KiB out-DMA spread over
    64 partitions (16 SDMA engines, small pipelined packets).

Both decodes multiply each exp term by a mean bias the host removes from
log(row_sum): KAPPA_SCH = ln(E[(1+u)2^-u]) for the Schraudolph mantissa
approximation, KAPPA_U8 = ln(sinh(h)/h), h = ln2/16, for the u8 grid. The
residual per-row ripple is zero-mean and ~0.3% — far inside the 2e-2
relative tolerance (measured end-to-end error ~1e-6).
"""

import sys

import numpy as np

if "/opt/trn_rl_repo" not in sys.path:
    sys.path.insert(0, "/opt/trn_rl_repo")

N_CORES = 8
N = 262144
C = 128  # classes
M = N // N_CORES  # rows per core (32768)
P = 128  # SBUF partitions
BLK = 512  # rows per matmul (PSUM bank = 512 fp32)
NB = M // BLK  # 64 blocks -> PSUM partitions
ALPHA = 0.5
BETA = 0.5
EPS = 1e-9

# Tile table: name -> (tag, rows). STREAM gives the DMA issue/landing order
# (U tiles front-loaded, first one small, so ScalarE starts early and never
# starves). KORDER gives the row-block order: each tile covers the next
# `rows` rows of the shard in KORDER. Decoupling the two lets the last
# ACT-decoded tile (U4) sit near the end of the block order while its data
# lands mid-stream, so TensorE's post-ACT tail is only the two tiny F tiles
# that the DVE decodes in ~0.2us each.
TILE = {
    "U0": ("U", 2048),
    "U1": ("U", 8192),
    "U2": ("U", 4096),
    "U3": ("U", 2048),
    "F0": ("F", 4096),
    "F1": ("F", 4096),
    "F2": ("F", 4096),
    "F3": ("F", 2048),
    "F4": ("F", 2048),
}
STREAM = ["U0", "U1", "F0", "U2", "F1", "U3", "F2", "F3", "F4"]
KORDER = ["U0", "U1", "F0", "F1", "F2", "F3", "U2", "U3", "F4"]
RU = sum(sz for tag, sz in TILE.values() if tag == "U")  # 16384
RF = sum(sz for tag, sz in TILE.values() if tag == "F")  # 16384

LOG2E = 1.4426950408889634
_LN2 = 0.6931471805599453

# Schraudolph (F path): bits16 = x * 1024*log2(e) + 15360.
A_SCH = 1024.0 * LOG2E
B_SCH = 15360.0
# ln(E[(1+u) 2^-u]) for u ~ U[0,1): mean log bias of the mantissa approx.
KAPPA_SCH = float(np.log(0.5 / _LN2 + (1.0 - (1.0 + _LN2) * 0.5) / (_LN2 * _LN2)))

# u8 grid (U path): ACT computes exp(u * ln2/8 - 16 ln2) = e^x on the grid.
S_U8 = _LN2 / 8.0
B_U8 = -16.0 * _LN2
_H = _LN2 / 16.0
KAPPA_U8 = float(np.log(np.sinh(_H) / _H))

_CACHE: dict = {}


def _build_nc_v5():
    import concourse.bacc as bacc
    import concourse.mybir as mybir

    f16 = mybir.dt.float16
    i16 = mybir.dt.int16
    u8 = mybir.dt.uint8
    f32 = mybir.dt.float32
    mult = mybir.AluOpType.mult
    add = mybir.AluOpType.add
    Exp = mybir.ActivationFunctionType.Exp

    nc = bacc.Bacc(
        "TRN2", target_bir_lowering=False, debug=False, num_devices=N_CORES
    )
    yU = nc.dram_tensor("y_u8", [C, RU], u8, kind="ExternalInput").ap()
    yF = nc.dram_tensor("y_f16", [C, RF], f16, kind="ExternalInput").ap()
    out = nc.dram_tensor("out", [NB, BLK], f16, kind="ExternalOutput").ap()

    n_tiles = len(STREAM)
    # per-path running column offsets into yU / yF, in stream order
    srcs = {}
    ou = of = 0
    for name in STREAM:
        tag, sz = TILE[name]
        if tag == "U":
            srcs[name] = (yU, ou, sz)
            ou += sz
        else:
            srcs[name] = (yF, of, sz)
            of += sz
    # decode-completion sem target per tile: 1 + position among same-tag
    # tiles in stream (=engine processing) order
    sem_tgt = {}
    ua = fa = 0
    for name in STREAM:
        if TILE[name][0] == "U":
            ua += 1
            sem_tgt[name] = ua
        else:
            fa += 1
            sem_tgt[name] = fa
    dsem_idx = {name: i for i, name in enumerate(STREAM)}
    # first block index of each tile, in KORDER
    kstart = {}
    k = 0
    for name in KORDER:
        kstart[name] = k
        k += TILE[name][1] // BLK

    T_s = {
        name: nc.alloc_sbuf_tensor(
            f"T{name}", [P, TILE[name][1]], u8 if TILE[name][0] == "U" else f16
        )
        for name in STREAM
    }
    E_s = {
        name: nc.alloc_sbuf_tensor(
            f"E{name}", [P, TILE[name][1]], f16 if TILE[name][0] == "U" else i16
        )
        for name in STREAM
    }
    # Shifted-window stationary: zeros except absolute column NB(=64) = ones,
    # so W[:, NB-k:2NB-k][:, j] is one iff j == k and block k's matmul writes
    # only PSUM partition k (others accumulate +0).
    W = nc.alloc_sbuf_tensor("W", [P, 2 * NB], f16)
    obuf = nc.alloc_sbuf_tensor("obuf", [NB, BLK], f16)
    scratch = nc.alloc_sbuf_tensor("scratch", [P, 8], f16)
    biasU = nc.alloc_sbuf_tensor("biasU", [P, 1], f32)
    ps = nc.alloc_psum_tensor("ps", [NB, BLK], f32)

    import contextlib

    with contextlib.ExitStack() as stack:
        block = stack.enter_context(nc.Block())
        # A dma's then_inc(sem, 16) arrives as 16 independent +1s (one per
        # SDMA slot), so each tile gets its own semaphore.
        dsem = [
            stack.enter_context(nc.semaphore(f"s_dma{i}")) for i in range(n_tiles)
        ]
        s_w = stack.enter_context(nc.semaphore("s_w"))
        s_fexp = stack.enter_context(nc.semaphore("s_fexp"))
        s_aexp = stack.enter_context(nc.semaphore("s_aexp"))
        s_mm = stack.enter_context(nc.semaphore("s_mm"))
        s_cp = stack.enter_context(nc.semaphore("s_cp"))
        s_out = stack.enter_context(nc.semaphore("s_out"))
        all_sems = dsem + [s_w, s_fexp, s_aexp, s_mm, s_cp, s_out]
        sem_nums = sorted(s.num for s in all_sems)

        @block.sync
        def _(sync):
            for i, name in enumerate(STREAM):
                src, off, sz = srcs[name]
                sync.dma_start(
                    out=T_s[name].ap(), in_=src[:, off : off + sz]
                ).then_inc(dsem[i], 16)
            sync.wait_ge(s_cp, 1)
            sync.dma_start(out=out[:], in_=obuf.ap()).then_inc(s_out, 16)
            sync.wait_ge(s_out, 16)
            # Re-execution safety: reset DMA bookkeeping and zero the sems.
            sync.drain(semaphore_range=range(sem_nums[0], sem_nums[-1] + 1))
            sync.sem_clear(range(sem_nums[0], sem_nums[-1] + 1))

        @block.gpsimd
        def _(g):
            g.memset(W.ap(), 0.0)
            g.memset(biasU.ap(), B_U8)
            g.memset(W.ap()[:, NB : NB + 1], 1.0).then_inc(s_w, 1)

        # ScalarE: warm-up ACT first (hoists the ~1.3us exp table load to
        # block start, under the first DMA), then one Exp per U tile; the
        # instruction's free affine does the u8 grid decode.
        @block.scalar
        def _(sc):
            sc.activation(scratch.ap(), scratch.ap(), Exp)
            sc.wait_ge(s_w, 1)
            for name in STREAM:
                if TILE[name][0] != "U":
                    continue
                sc.wait_ge(dsem[dsem_idx[name]], 16)
                sc.activation(
                    E_s[name].ap(), T_s[name].ap(), Exp, bias=biasU.ap(), scale=S_U8
                ).then_inc(s_aexp, 1)

        # DVE: one tensor_scalar per F tile (fp16 in, int16 out, 4x mode),
        # then the single PSUM->SBUF f32->f16 copy of the [64, 512] image.
        @block.vector
        def _(v):
            for name in STREAM:
                if TILE[name][0] != "F":
                    continue
                v.wait_ge(dsem[dsem_idx[name]], 16)
                v.tensor_scalar(
                    E_s[name].ap(), T_s[name].ap(), A_SCH, B_SCH, mult, add
                ).then_inc(s_fexp, 1)
            v.wait_ge(s_mm, NB)
            v.tensor_copy(obuf.ap(), ps.ap()).then_inc(s_cp, 1)

        @block.tensor
        def _(t):
            t.wait_ge(s_w, 1)
            for name in KORDER:
                tag, sz = TILE[name]
                t.wait_ge(s_aexp if tag == "U" else s_fexp, sem_tgt[name])
                E16 = E_s[name].ap()
                if tag == "F":
                    E16 = E16.bitcast(f16)
                for j in range(sz // BLK):
                    k = kstart[name] + j
                    t.matmul(
                        ps.ap()[:, :],
                        W.ap()[:, NB - k : 2 * NB - k],
                        E16[:, j * BLK : (j + 1) * BLK],
                        start=(k == 0),
                        stop=(k == NB - 1),
                    ).then_inc(s_mm, 1)

    nc.finalize()
    return nc


def _get_nc():
    if "nc" not in _CACHE:
        _CACHE["nc"] = _build_nc_v5()
    return _CACHE["nc"]


def _row_ranges() -> dict:
    """Tile -> (row_start, rows) of the core shard, per KORDER."""
    rr = {}
    off = 0
    for name in KORDER:
        sz = TILE[name][1]
        rr[name] = (off, sz)
        off += sz
    return rr


def _make_in_maps(y_pred: np.ndarray):
    yp = np.asarray(y_pred)
    rr = _row_ranges()
    maps = []
    for c in range(N_CORES):
        sh = yp[c * M : (c + 1) * M]  # [M, C] fp32
        uC, fC = [], []
        for name in STREAM:
            tag, sz = TILE[name]
            off, _ = rr[name]
            rows = sh[off : off + sz]
            if tag == "U":
                uC.append(
                    np.clip(np.rint((rows * LOG2E + 16.0) * 8.0), 0.0, 255.0)
                    .astype(np.uint8)
                    .T
                )
            else:
                fC.append(rows.astype(np.float16).T)
        maps.append(
            {
                "y_u8": np.ascontiguousarray(np.concatenate(uC, axis=1)),
                "y_f16": np.ascontiguousarray(np.concatenate(fC, axis=1)),
            }
        )
    return maps


def _run(in_maps, trace=False, **kwargs):
    from concourse.bass_utils import run_bass_kernel_spmd

    nc = _get_nc()
    return run_bass_kernel_spmd(
        nc, in_maps, list(range(N_CORES)), trace=trace, **kwargs
    )


def _kappa_rows() -> np.ndarray:
    k = np.empty(M, dtype=np.float64)
    off = 0
    for name in KORDER:
        tag, sz = TILE[name]
        k[off : off + sz] = KAPPA_U8 if tag == "U" else KAPPA_SCH
        off += sz
    return k


def _combine(results, y_pred: np.ndarray, y_true: np.ndarray) -> np.ndarray:
    yp = np.asarray(y_pred)
    yt = np.asarray(y_true).reshape(-1).astype(np.int64)

    # Device out[k, c] = sum of block k (rows 512k..512k+511), already in
    # row order; subtract the per-path mean log bias.
    kap = _kappa_rows()
    lse = np.empty(N, dtype=np.float64)
    for c in range(N_CORES):
        sums = results[c]["out"].astype(np.float64).reshape(M)
        lse[c * M : (c + 1) * M] = np.log(sums) - kap

    picked = np.take_along_axis(yp, yt[:, None], axis=1).reshape(-1).astype(np.float64)
    ce = -(picked.sum() - lse.sum()) / N

    p1 = np.exp(yp[:, 0].astype(np.float64) - lse)
    lp = np.log(p1 + EPS)
    lq = np.log((1.0 + EPS) - p1)
    nj = np.bincount(yt, minlength=C).astype(np.float64)
    s = BETA * (1.0 - nj / (N - nj[0]))
    v = np.where(yt == 0, ALPHA * lp, s[yt] * lq)
    loss = ce - v.sum() / N
    return np.asarray(loss, dtype=np.float32)


def kernel(y_pred: np.ndarray, y_true: np.ndarray) -> np.ndarray:
    in_maps = _make_in_maps(y_pred)
    res = _run(in_maps, trace=False)
    return _combine(res.results, y_pred, y_true)


# revision 28
# speedup vs baseline: 1.0452x; 1.0452x over previous
"""Trainium2 Bass kernel for nn_CustomLoss_19061064859882.

loss = CE(y_pred, y_true) - penalty/N, where the penalty uses
p1 = softmax(y_pred)[:, 0] and per-class weights from the label histogram.

Device/host split: the O(N*C) work — per-row sum_c exp(y_pred[r, c]) over the
128 classes — runs on the 8 NeuronCores (data-parallel over rows). The
remaining O(N) bookkeeping (log of the row-sums, picked-logit gather, label
bincount, per-class weighted sums, final scalar) is cheap vectorized numpy on
the host in float64.

Device pipeline (v5) — DMA-bound, dual-engine decode:
  * Each core's shard ships TRANSPOSED (classes on partitions, rows on the
    free dim) and split across two wire formats to cut HBM bytes ~25%:
      - U tiles (16384 rows): uint8 codes u = round((x*log2e + 16)*8), i.e.
        the logit on a 1/8-log2 grid. The otherwise-idle ScalarE decodes
        them with a single ACT pass: exp(u*ln2/8 - 16*ln2) = e^x, using the
        ACT instruction's free scale/bias affine. 1 elem/lane/cycle.
      - F tiles (16384 rows): fp16 logits decoded on the DVE via the
        Schraudolph bit trick at 4x perf mode: tensor_scalar computes
        i16 = int(x*1024*log2(e) + 15360); those bits reinterpreted as fp16
        are 2^(x*log2e).
    U and F tiles interleave in the DMA stream so both engines stay fed.
  * class-sum via TensorE: ones-column matmuls reduce over the partition
    (class) dim, 512 rows per matmul into PSUM fp32. A shifted-window
    stationary (W[:, 64-k:128-k], ones only in absolute column 64) routes
    512-row block k to PSUM partition k, all accumulating into one
    [64, 512] PSUM bank.
  * one DVE PSUM->SBUF copy (fp32 -> fp16), one 64KiB out-DMA spread over
    64 partitions (16 SDMA engines, small pipelined packets).

Both decodes multiply each exp term by a mean bias the host removes from
log(row_sum): KAPPA_SCH = ln(E[(1+u)2^-u]) for the Schraudolph mantissa
approximation, KAPPA_U8 = ln(sinh(h)/h), h = ln2/16, for the u8 grid. The
residual per-row ripple is zero-mean and ~0.3% — far inside the 2e-2
relative tolerance (measured end-to-end error ~1e-6).
"""

import sys

import numpy as np

if "/opt/trn_rl_repo" not in sys.path:
    sys.path.insert(0, "/opt/trn_rl_repo")

N_CORES = 8
N = 262144
C = 128  # classes
M = N // N_CORES  # rows per core (32768)
P = 128  # SBUF partitions
BLK = 512  # rows per matmul (PSUM bank = 512 fp32)
NB = M // BLK  # 64 blocks -> PSUM partitions
ALPHA = 0.5
BETA = 0.5
EPS = 1e-9

# Tile table: name -> (tag, rows). STREAM gives the DMA issue/landing order
# (U tiles front-loaded, first one small, so ScalarE starts early and never
# starves). KORDER gives the row-block order: each tile covers the next
# `rows` rows of the shard in KORDER. Decoupling the two lets the last
# ACT-decoded tile (U4) sit near the end of the block order while its data
# lands mid-stream, so TensorE's post-ACT tail is only the two tiny F tiles
# that the DVE decodes in ~0.2us each.
TILE = {
    "U0": ("U", 2048),
    "U1": ("U", 4096),
    "U2": ("U", 4096),
    "U3": ("U", 4096),
    "U4": ("U", 2048),
    "F0": ("F", 4096),
    "F1": ("F", 4096),
    "F2": ("F", 4096),
    "F3": ("F", 2048),
    "F4": ("F", 2048),
}
STREAM = ["U0", "U1", "F0", "U2", "F1", "U3", "F2", "U4", "F3", "F4"]
KORDER = ["U0", "U1", "F0", "F1", "F2", "U2", "F3", "U3", "U4", "F4"]
RU = sum(sz for tag, sz in TILE.values() if tag == "U")  # 16384
RF = sum(sz for tag, sz in TILE.values() if tag == "F")  # 16384

LOG2E = 1.4426950408889634
_LN2 = 0.6931471805599453

# Schraudolph (F path): bits16 = x * 1024*log2(e) + 15360.
A_SCH = 1024.0 * LOG2E
B_SCH = 15360.0
# ln(E[(1+u) 2^-u]) for u ~ U[0,1): mean log bias of the mantissa approx.
KAPPA_SCH = float(np.log(0.5 / _LN2 + (1.0 - (1.0 + _LN2) * 0.5) / (_LN2 * _LN2)))

# u8 grid (U path): ACT computes exp(u * ln2/8 - 16 ln2 - SHIFT) = e^(x-SHIFT)
# on the grid and writes fp8e4m3 (halves the SBUF write + PE read traffic).
# The -2 nat shift keeps values in [~2e-6, ~70], clear of both the fp8 max
# (240/448 depending on variant) and meaningful-underflow territory.
S_U8 = _LN2 / 8.0
SHIFT_U = 2.0
B_U8 = -16.0 * _LN2 - SHIFT_U
# mean log bias of (u8 grid + fp8 value quantization), fitted numerically
# (variant-agnostic to 2e-5); lse = log(sum) + SHIFT_U - 0.008243...
KAPPA_U8 = -0.008243 - SHIFT_U

_CACHE: dict = {}


def _build_nc_v5():
    import concourse.bacc as bacc
    import concourse.mybir as mybir

    f16 = mybir.dt.float16
    i16 = mybir.dt.int16
    u8 = mybir.dt.uint8
    f8 = mybir.dt.float8e4
    f32 = mybir.dt.float32
    mult = mybir.AluOpType.mult
    add = mybir.AluOpType.add
    Exp = mybir.ActivationFunctionType.Exp

    nc = bacc.Bacc(
        "TRN2", target_bir_lowering=False, debug=False, num_devices=N_CORES
    )
    yU = nc.dram_tensor("y_u8", [C, RU], u8, kind="ExternalInput").ap()
    yF = nc.dram_tensor("y_f16", [C, RF], f16, kind="ExternalInput").ap()
    out = nc.dram_tensor("out", [NB, BLK], f16, kind="ExternalOutput").ap()

    n_tiles = len(STREAM)
    # per-path running column offsets into yU / yF, in stream order
    srcs = {}
    ou = of = 0
    for name in STREAM:
        tag, sz = TILE[name]
        if tag == "U":
            srcs[name] = (yU, ou, sz)
            ou += sz
        else:
            srcs[name] = (yF, of, sz)
            of += sz
    # decode-completion sem target per tile: 1 + position among same-tag
    # tiles in stream (=engine processing) order
    sem_tgt = {}
    ua = fa = 0
    for name in STREAM:
        if TILE[name][0] == "U":
            ua += 1
            sem_tgt[name] = ua
        else:
            fa += 1
            sem_tgt[name] = fa
    dsem_idx = {name: i for i, name in enumerate(STREAM)}
    # first block index of each tile, in KORDER
    kstart = {}
    k = 0
    for name in KORDER:
        kstart[name] = k
        k += TILE[name][1] // BLK

    T_s = {
        name: nc.alloc_sbuf_tensor(
            f"T{name}", [P, TILE[name][1]], u8 if TILE[name][0] == "U" else f16
        )
        for name in STREAM
    }
    E_s = {
        name: nc.alloc_sbuf_tensor(
            f"E{name}", [P, TILE[name][1]], f8 if TILE[name][0] == "U" else i16
        )
        for name in STREAM
    }
    # Shifted-window stationary: zeros except absolute column NB(=64) = ones,
    # so W[:, NB-k:2NB-k][:, j] is one iff j == k and block k's matmul writes
    # only PSUM partition k (others accumulate +0).
    W = nc.alloc_sbuf_tensor("W", [P, 2 * NB], f16)
    obuf = nc.alloc_sbuf_tensor("obuf", [NB, BLK], f16)
    scratch = nc.alloc_sbuf_tensor("scratch", [P, 8], f16)
    biasU = nc.alloc_sbuf_tensor("biasU", [P, 1], f32)
    ps = nc.alloc_psum_tensor("ps", [NB, BLK], f32)

    import contextlib

    with contextlib.ExitStack() as stack:
        block = stack.enter_context(nc.Block())
        # A dma's then_inc(sem, 16) arrives as 16 independent +1s (one per
        # SDMA slot), so each tile gets its own semaphore.
        dsem = [
            stack.enter_context(nc.semaphore(f"s_dma{i}")) for i in range(n_tiles)
        ]
        s_w = stack.enter_context(nc.semaphore("s_w"))
        s_fexp = stack.enter_context(nc.semaphore("s_fexp"))
        s_aexp = stack.enter_context(nc.semaphore("s_aexp"))
        s_mm = stack.enter_context(nc.semaphore("s_mm"))
        s_cp = stack.enter_context(nc.semaphore("s_cp"))
        s_out = stack.enter_context(nc.semaphore("s_out"))
        all_sems = dsem + [s_w, s_fexp, s_aexp, s_mm, s_cp, s_out]
        sem_nums = sorted(s.num for s in all_sems)

        @block.sync
        def _(sync):
            for i, name in enumerate(STREAM):
                if TILE[name][0] != "U":
                    continue
                src, off, sz = srcs[name]
                sync.dma_start(
                    out=T_s[name].ap(), in_=src[:, off : off + sz]
                ).then_inc(dsem[i], 16)
            sync.wait_ge(s_cp, 1)
            sync.dma_start(out=out[:], in_=obuf.ap()).then_inc(s_out, 16)
            sync.wait_ge(s_out, 16)
            # Re-execution safety: reset DMA bookkeeping and zero the sems.
            sync.drain(semaphore_range=range(sem_nums[0], sem_nums[-1] + 1))
            sync.sem_clear(range(sem_nums[0], sem_nums[-1] + 1))

        @block.gpsimd
        def _(g):
            g.memset(W.ap(), 0.0)
            g.memset(biasU.ap(), B_U8)
            g.memset(W.ap()[:, NB : NB + 1], 1.0).then_inc(s_w, 1)

        # ScalarE: warm-up ACT first (hoists the ~1.3us exp table load to
        # block start, under the first DMA), then one Exp per U tile; the
        # instruction's free affine does the u8 grid decode.
        @block.scalar
        def _(sc):
            sc.activation(scratch.ap(), scratch.ap(), Exp)
            # F-tile DMAs ride the second (ACT) HWDGE ring so the two rings
            # generate + drain descriptors in parallel with the sync ring's
            # U tiles, tightening the stream ramp.
            for name in STREAM:
                if TILE[name][0] != "F":
                    continue
                src, off, sz = srcs[name]
                sc.dma_start(
                    out=T_s[name].ap(), in_=src[:, off : off + sz]
                ).then_inc(dsem[dsem_idx[name]], 16)
            sc.wait_ge(s_w, 1)
            for name in STREAM:
                if TILE[name][0] != "U":
                    continue
                sc.wait_ge(dsem[dsem_idx[name]], 16)
                sc.activation(
                    E_s[name].ap(), T_s[name].ap(), Exp, bias=biasU.ap(), scale=S_U8
                ).then_inc(s_aexp, 1)

        # DVE: one tensor_scalar per F tile (fp16 in, int16 out, 4x mode),
        # then the single PSUM->SBUF f32->f16 copy of the [64, 512] image.
        @block.vector
        def _(v):
            for name in STREAM:
                if TILE[name][0] != "F":
                    continue
                v.wait_ge(dsem[dsem_idx[name]], 16)
                v.tensor_scalar(
                    E_s[name].ap(), T_s[name].ap(), A_SCH, B_SCH, mult, add
                ).then_inc(s_fexp, 1)
            v.wait_ge(s_mm, NB)
            v.tensor_copy(obuf.ap(), ps.ap()).then_inc(s_cp, 1)

        @block.tensor
        def _(t):
            t.wait_ge(s_w, 1)
            for name in KORDER:
                tag, sz = TILE[name]
                t.wait_ge(s_aexp if tag == "U" else s_fexp, sem_tgt[name])
                E16 = E_s[name].ap()
                if tag == "F":
                    E16 = E16.bitcast(f16)
                for j in range(sz // BLK):
                    k = kstart[name] + j
                    t.matmul(
                        ps.ap()[:, :],
                        W.ap()[:, NB - k : 2 * NB - k],
                        E16[:, j * BLK : (j + 1) * BLK],
                        start=(k == 0),
                        stop=(k == NB - 1),
                    ).then_inc(s_mm, 1)

    nc.finalize()
    return nc


def _get_nc():
    if "nc" not in _CACHE:
        _CACHE["nc"] = _build_nc_v5()
    return _CACHE["nc"]


def _row_ranges() -> dict:
    """Tile -> (row_start, rows) of the core shard, per KORDER."""
    rr = {}
    off = 0
    for name in KORDER:
        sz = TILE[name][1]
        rr[name] = (off, sz)
        off += sz
    return rr


def _make_in_maps(y_pred: np.ndarray):
    yp = np.asarray(y_pred)
    rr = _row_ranges()
    maps = []
    for c in range(N_CORES):
        sh = yp[c * M : (c + 1) * M]  # [M, C] fp32
        uC, fC = [], []
        for name in STREAM:
            tag, sz = TILE[name]
            off, _ = rr[name]
            rows = sh[off : off + sz]
            if tag == "U":
                uC.append(
                    np.clip(np.rint((rows * LOG2E + 16.0) * 8.0), 0.0, 255.0)
                    .astype(np.uint8)
                    .T
                )
            else:
                fC.append(rows.astype(np.float16).T)
        maps.append(
            {
                "y_u8": np.ascontiguousarray(np.concatenate(uC, axis=1)),
                "y_f16": np.ascontiguousarray(np.concatenate(fC, axis=1)),
            }
        )
    return maps


def _run(in_maps, trace=False, **kwargs):
    from concourse.bass_utils import run_bass_kernel_spmd

    nc = _get_nc()
    return run_bass_kernel_spmd(
        nc, in_maps, list(range(N_CORES)), trace=trace, **kwargs
    )


def _kappa_rows() -> np.ndarray:
    k = np.empty(M, dtype=np.float64)
    off = 0
    for name in KORDER:
        tag, sz = TILE[name]
        k[off : off + sz] = KAPPA_U8 if tag == "U" else KAPPA_SCH
        off += sz
    return k


def _combine(results, y_pred: np.ndarray, y_true: np.ndarray) -> np.ndarray:
    yp = np.asarray(y_pred)
    yt = np.asarray(y_true).reshape(-1).astype(np.int64)

    # Device out[k, c] = sum of block k (rows 512k..512k+511), already in
    # row order; subtract the per-path mean log bias.
    kap = _kappa_rows()
    lse = np.empty(N, dtype=np.float64)
    for c in range(N_CORES):
        sums = results[c]["out"].astype(np.float64).reshape(M)
        lse[c * M : (c + 1) * M] = np.log(sums) - kap

    picked = np.take_along_axis(yp, yt[:, None], axis=1).reshape(-1).astype(np.float64)
    ce = -(picked.sum() - lse.sum()) / N

    p1 = np.exp(yp[:, 0].astype(np.float64) - lse)
    lp = np.log(p1 + EPS)
    lq = np.log((1.0 + EPS) - p1)
    nj = np.bincount(yt, minlength=C).astype(np.float64)
    s = BETA * (1.0 - nj / (N - nj[0]))
    v = np.where(yt == 0, ALPHA * lp, s[yt] * lq)
    loss = ce - v.sum() / N
    return np.asarray(loss, dtype=np.float32)


def kernel(y_pred: np.ndarray, y_true: np.ndarray) -> np.ndarray:
    in_maps = _make_in_maps(y_pred)
    res = _run(in_maps, trace=False)
    return _combine(res.results, y_pred, y_true)


# revision 30
# speedup vs baseline: 1.0744x; 1.0279x over previous
"""Trainium2 Bass kernel for nn_CustomLoss_19061064859882.

loss = CE(y_pred, y_true) - penalty/N, where the penalty uses
p1 = softmax(y_pred)[:, 0] and per-class weights from the label histogram.

Device/host split: the O(N*C) work — per-row sum_c exp(y_pred[r, c]) over the
128 classes — runs on the 8 NeuronCores (data-parallel over rows). The
remaining O(N) bookkeeping (log of the row-sums, picked-logit gather, label
bincount, per-class weighted sums, final scalar) is cheap vectorized numpy on
the host in float64.

Device pipeline (v5) — DMA-bound, dual-engine decode:
  * Each core's shard ships TRANSPOSED (classes on partitions, rows on the
    free dim) and split across two wire formats to cut HBM bytes ~25%:
      - U tiles (16384 rows): uint8 codes u = round((x*log2e + 16)*8), i.e.
        the logit on a 1/8-log2 grid. The otherwise-idle ScalarE decodes
        them with a single ACT pass: exp(u*ln2/8 - 16*ln2) = e^x, using the
        ACT instruction's free scale/bias affine. 1 elem/lane/cycle.
      - F tiles (16384 rows): fp16 logits decoded on the DVE via the
        Schraudolph bit trick at 4x perf mode: tensor_scalar computes
        i16 = int(x*1024*log2(e) + 15360); those bits reinterpreted as fp16
        are 2^(x*log2e).
    U and F tiles interleave in the DMA stream so both engines stay fed.
  * class-sum via TensorE: ones-column matmuls reduce over the partition
    (class) dim, 512 rows per matmul into PSUM fp32. A shifted-window
    stationary (W[:, 64-k:128-k], ones only in absolute column 64) routes
    512-row block k to PSUM partition k, all accumulating into one
    [64, 512] PSUM bank.
  * one DVE PSUM->SBUF copy (fp32 -> fp16), one 64KiB out-DMA spread over
    64 partitions (16 SDMA engines, small pipelined packets).

Both decodes multiply each exp term by a mean bias the host removes from
log(row_sum): KAPPA_SCH = ln(E[(1+u)2^-u]) for the Schraudolph mantissa
approximation, KAPPA_U8 = ln(sinh(h)/h), h = ln2/16, for the u8 grid. The
residual per-row ripple is zero-mean and ~0.3% — far inside the 2e-2
relative tolerance (measured end-to-end error ~1e-6).
"""

import sys

import numpy as np

if "/opt/trn_rl_repo" not in sys.path:
    sys.path.insert(0, "/opt/trn_rl_repo")

N_CORES = 8
N = 262144
C = 128  # classes
M = N // N_CORES  # rows per core (32768)
P = 128  # SBUF partitions
BLK = 512  # rows per matmul (PSUM bank = 512 fp32)
NB = M // BLK  # 64 blocks -> PSUM partitions
ALPHA = 0.5
BETA = 0.5
EPS = 1e-9

# Tile table: name -> (tag, rows). STREAM gives the DMA issue/landing order
# (U tiles front-loaded, first one small, so ScalarE starts early and never
# starves). KORDER gives the row-block order: each tile covers the next
# `rows` rows of the shard in KORDER. Decoupling the two lets the last
# ACT-decoded tile (U4) sit near the end of the block order while its data
# lands mid-stream, so TensorE's post-ACT tail is only the two tiny F tiles
# that the DVE decodes in ~0.2us each.
TILE = {
    "U0": ("U", 2048),
    "U1": ("U", 4096),
    "U2": ("U", 4096),
    "U3": ("U", 4096),
    "U4": ("U", 2048),
    "F0": ("F", 4096),
    "F1": ("F", 4096),
    "F2": ("F", 4096),
    "F3": ("F", 2048),
    "F4": ("F", 2048),
}
STREAM = ["U0", "U1", "F0", "U2", "F1", "U3", "F2", "U4", "F3", "F4"]
KORDER = ["U0", "U1", "F0", "F1", "F2", "U2", "F3", "U3", "U4", "F4"]
RU = sum(sz for tag, sz in TILE.values() if tag == "U")  # 16384
RF = sum(sz for tag, sz in TILE.values() if tag == "F")  # 16384

LOG2E = 1.4426950408889634
_LN2 = 0.6931471805599453

# Schraudolph (F path): bits16 = x * 1024*log2(e) + 15360.
A_SCH = 1024.0 * LOG2E
B_SCH = 15360.0
# ln(E[(1+u) 2^-u]) for u ~ U[0,1): mean log bias of the mantissa approx.
KAPPA_SCH = float(np.log(0.5 / _LN2 + (1.0 - (1.0 + _LN2) * 0.5) / (_LN2 * _LN2)))

# u8 grid (U path): ACT computes exp(u * ln2/8 - 16 ln2 - SHIFT) = e^(x-SHIFT)
# on the grid and writes fp8e4m3 (halves the SBUF write + PE read traffic).
# The -2 nat shift keeps values in [~2e-6, ~70], clear of both the fp8 max
# (240/448 depending on variant) and meaningful-underflow territory.
S_U8 = _LN2 / 8.0
SHIFT_U = 2.0
B_U8 = -16.0 * _LN2 - SHIFT_U
# mean log bias of (u8 grid + fp8 value quantization), fitted numerically
# (variant-agnostic to 2e-5); lse = log(sum) + SHIFT_U - 0.008243...
KAPPA_U8 = -0.008243 - SHIFT_U

_CACHE: dict = {}


def _build_nc_v5():
    import concourse.bacc as bacc
    import concourse.mybir as mybir

    f16 = mybir.dt.float16
    i16 = mybir.dt.int16
    u8 = mybir.dt.uint8
    f8 = mybir.dt.float8e4
    f32 = mybir.dt.float32
    mult = mybir.AluOpType.mult
    add = mybir.AluOpType.add
    Exp = mybir.ActivationFunctionType.Exp

    nc = bacc.Bacc(
        "TRN2", target_bir_lowering=False, debug=False, num_devices=N_CORES
    )
    yU = nc.dram_tensor("y_u8", [C, RU], u8, kind="ExternalInput").ap()
    yF = nc.dram_tensor("y_f16", [C, RF], f16, kind="ExternalInput").ap()
    out = nc.dram_tensor("out", [NB, BLK], f16, kind="ExternalOutput").ap()

    n_tiles = len(STREAM)
    # per-path running column offsets into yU / yF, in stream order
    srcs = {}
    ou = of = 0
    for name in STREAM:
        tag, sz = TILE[name]
        if tag == "U":
            srcs[name] = (yU, ou, sz)
            ou += sz
        else:
            srcs[name] = (yF, of, sz)
            of += sz
    # decode-completion sem target per tile: 1 + position among same-tag
    # tiles in stream (=engine processing) order
    sem_tgt = {}
    ua = fa = 0
    for name in STREAM:
        if TILE[name][0] == "U":
            ua += 1
            sem_tgt[name] = ua
        else:
            fa += 1
            sem_tgt[name] = fa
    dsem_idx = {name: i for i, name in enumerate(STREAM)}
    # first block index of each tile, in KORDER
    kstart = {}
    k = 0
    for name in KORDER:
        kstart[name] = k
        k += TILE[name][1] // BLK

    T_s = {
        name: nc.alloc_sbuf_tensor(
            f"T{name}", [P, TILE[name][1]], u8 if TILE[name][0] == "U" else f16
        )
        for name in STREAM
    }
    E_s = {
        name: nc.alloc_sbuf_tensor(
            f"E{name}", [P, TILE[name][1]], f8 if TILE[name][0] == "U" else i16
        )
        for name in STREAM
    }
    # Shifted-window stationary: zeros except absolute column NB(=64) = ones,
    # so W[:, NB-k:2NB-k][:, j] is one iff j == k and block k's matmul writes
    # only PSUM partition k (others accumulate +0).
    W = nc.alloc_sbuf_tensor("W", [P, 2 * NB], f16)
    obuf = nc.alloc_sbuf_tensor("obuf", [NB, BLK], f16)
    scratch = nc.alloc_sbuf_tensor("scratch", [P, 8], f16)
    biasU = nc.alloc_sbuf_tensor("biasU", [P, 1], f32)
    ps = nc.alloc_psum_tensor("ps", [NB, BLK], f32)

    import contextlib

    with contextlib.ExitStack() as stack:
        block = stack.enter_context(nc.Block())
        # A dma's then_inc(sem, 16) arrives as 16 independent +1s (one per
        # SDMA slot), so each tile gets its own semaphore.
        dsem = [
            stack.enter_context(nc.semaphore(f"s_dma{i}")) for i in range(n_tiles)
        ]
        s_w = stack.enter_context(nc.semaphore("s_w"))
        s_fexp = stack.enter_context(nc.semaphore("s_fexp"))
        s_aexp = stack.enter_context(nc.semaphore("s_aexp"))
        s_mm = stack.enter_context(nc.semaphore("s_mm"))
        s_cp = stack.enter_context(nc.semaphore("s_cp"))
        s_out = stack.enter_context(nc.semaphore("s_out"))
        all_sems = dsem + [s_w, s_fexp, s_aexp, s_mm, s_cp, s_out]
        sem_nums = sorted(s.num for s in all_sems)

        @block.sync
        def _(sync):
            # Single HWDGE ring on purpose: the SDMA engines round-robin
            # packets across rings, so a second ring would interleave tile
            # completions and starve the ACT decode chain of its U tiles.
            for i, name in enumerate(STREAM):
                src, off, sz = srcs[name]
                sync.dma_start(
                    out=T_s[name].ap(), in_=src[:, off : off + sz]
                ).then_inc(dsem[i], 16)
            sync.wait_ge(s_cp, 1)
            sync.dma_start(out=out[:], in_=obuf.ap()).then_inc(s_out, 16)
            sync.wait_ge(s_out, 16)
            # Re-execution safety: reset DMA bookkeeping and zero the sems.
            sync.drain(semaphore_range=range(sem_nums[0], sem_nums[-1] + 1))
            sync.sem_clear(range(sem_nums[0], sem_nums[-1] + 1))

        @block.gpsimd
        def _(g):
            g.memset(W.ap(), 0.0)
            g.memset(biasU.ap(), B_U8)
            g.memset(W.ap()[:, NB : NB + 1], 1.0).then_inc(s_w, 1)

        # ScalarE: warm-up ACT first (hoists the ~1.3us exp table load to
        # block start, under the first DMA), then one Exp per U tile; the
        # instruction's free affine does the u8 grid decode.
        @block.scalar
        def _(sc):
            sc.activation(scratch.ap(), scratch.ap(), Exp)
            sc.wait_ge(s_w, 1)
            for name in STREAM:
                if TILE[name][0] != "U":
                    continue
                sc.wait_ge(dsem[dsem_idx[name]], 16)
                sc.activation(
                    E_s[name].ap(), T_s[name].ap(), Exp, bias=biasU.ap(), scale=S_U8
                ).then_inc(s_aexp, 1)

        # DVE: one tensor_scalar per F tile (fp16 in, int16 out, 4x mode),
        # then the single PSUM->SBUF f32->f16 copy of the [64, 512] image.
        @block.vector
        def _(v):
            for name in STREAM:
                if TILE[name][0] != "F":
                    continue
                v.wait_ge(dsem[dsem_idx[name]], 16)
                v.tensor_scalar(
                    E_s[name].ap(), T_s[name].ap(), A_SCH, B_SCH, mult, add
                ).then_inc(s_fexp, 1)
            v.wait_ge(s_mm, NB)
            v.tensor_copy(obuf.ap(), ps.ap()).then_inc(s_cp, 1)

        @block.tensor
        def _(t):
            t.wait_ge(s_w, 1)
            for name in KORDER:
                tag, sz = TILE[name]
                t.wait_ge(s_aexp if tag == "U" else s_fexp, sem_tgt[name])
                E16 = E_s[name].ap()
                if tag == "F":
                    E16 = E16.bitcast(f16)
                for j in range(sz // BLK):
                    k = kstart[name] + j
                    t.matmul(
                        ps.ap()[:, :],
                        W.ap()[:, NB - k : 2 * NB - k],
                        E16[:, j * BLK : (j + 1) * BLK],
                        start=(k == 0),
                        stop=(k == NB - 1),
                    ).then_inc(s_mm, 1)

    nc.finalize()
    return nc


def _get_nc():
    if "nc" not in _CACHE:
        _CACHE["nc"] = _build_nc_v5()
    return _CACHE["nc"]


def _row_ranges() -> dict:
    """Tile -> (row_start, rows) of the core shard, per KORDER."""
    rr = {}
    off = 0
    for name in KORDER:
        sz = TILE[name][1]
        rr[name] = (off, sz)
        off += sz
    return rr


def _make_in_maps(y_pred: np.ndarray):
    yp = np.asarray(y_pred)
    rr = _row_ranges()
    maps = []
    for c in range(N_CORES):
        sh = yp[c * M : (c + 1) * M]  # [M, C] fp32
        uC, fC = [], []
        for name in STREAM:
            tag, sz = TILE[name]
            off, _ = rr[name]
            rows = sh[off : off + sz]
            if tag == "U":
                uC.append(
                    np.clip(np.rint((rows * LOG2E + 16.0) * 8.0), 0.0, 255.0)
                    .astype(np.uint8)
                    .T
                )
            else:
                fC.append(rows.astype(np.float16).T)
        maps.append(
            {
                "y_u8": np.ascontiguousarray(np.concatenate(uC, axis=1)),
                "y_f16": np.ascontiguousarray(np.concatenate(fC, axis=1)),
            }
        )
    return maps


def _run(in_maps, trace=False, **kwargs):
    from concourse.bass_utils import run_bass_kernel_spmd

    nc = _get_nc()
    return run_bass_kernel_spmd(
        nc, in_maps, list(range(N_CORES)), trace=trace, **kwargs
    )


def _kappa_rows() -> np.ndarray:
    k = np.empty(M, dtype=np.float64)
    off = 0
    for name in KORDER:
        tag, sz = TILE[name]
        k[off : off + sz] = KAPPA_U8 if tag == "U" else KAPPA_SCH
        off += sz
    return k


def _combine(results, y_pred: np.ndarray, y_true: np.ndarray) -> np.ndarray:
    yp = np.asarray(y_pred)
    yt = np.asarray(y_true).reshape(-1).astype(np.int64)

    # Device out[k, c] = sum of block k (rows 512k..512k+511), already in
    # row order; subtract the per-path mean log bias.
    kap = _kappa_rows()
    lse = np.empty(N, dtype=np.float64)
    for c in range(N_CORES):
        sums = results[c]["out"].astype(np.float64).reshape(M)
        lse[c * M : (c + 1) * M] = np.log(sums) - kap

    picked = np.take_along_axis(yp, yt[:, None], axis=1).reshape(-1).astype(np.float64)
    ce = -(picked.sum() - lse.sum()) / N

    p1 = np.exp(yp[:, 0].astype(np.float64) - lse)
    lp = np.log(p1 + EPS)
    lq = np.log((1.0 + EPS) - p1)
    nj = np.bincount(yt, minlength=C).astype(np.float64)
    s = BETA * (1.0 - nj / (N - nj[0]))
    v = np.where(yt == 0, ALPHA * lp, s[yt] * lq)
    loss = ce - v.sum() / N
    return np.asarray(loss, dtype=np.float32)


def kernel(y_pred: np.ndarray, y_true: np.ndarray) -> np.ndarray:
    in_maps = _make_in_maps(y_pred)
    res = _run(in_maps, trace=False)
    return _combine(res.results, y_pred, y_true)
